# revision 20
# baseline (speedup 1.0000x reference)
"""Trainium2 Bass kernel for nn_AMS_18975165514201 (moe_routing).

Data-parallel over batch B=64 on 8 NeuronCores (8 rows/core), no
collectives.  Rows are processed in PAIRS so the series dimension packs
the full 128 partitions (2 rows x 64 channels).  Per pair, on device:
  - stream both rows of x (2 x 2.75 MB) into SBUF
  - extract x3 = x[..., 0] with strided on-chip copies -> [112, 128] x3T
  - series_decomp_multi: one cumsum (tensor_tensor_scan) over an
    edge-padded tile, window sums by shifted subtraction, mixed by a
    softmax of decomp projections
  - Fourier layer: DFT via matmuls against cos/sin constants, per-series
    top-3 bin selection (InstMax + is_ge mask), masked reconstruction
  - gating GEMMs (block-diagonal start_w packs both rows) -> per-row
    top-2 softmax gate-sum s
  - scale the SBUF-resident rows by s and stream them out
The gate-sum s equals 1 to within 1e-7 for ANY noise_std (softmax rows
sum to 1), so the device uses std=1 for its own top-2 pick; "stats"
carries clean logits + wnpre per row and the host combine computes the
exact noise_std/importance/load/balance loss in float64.
"""

import math
from contextlib import ExitStack

import ml_dtypes
import numpy as np

import concourse.bass as bass
import concourse.tile as tile
from concourse import bacc, mybir
from concourse.bass_utils import run_bass_kernel_spmd
from concourse.masks import make_identity

F32 = mybir.dt.float32
BF16 = mybir.dt.bfloat16
AX = mybir.AxisListType
AF = mybir.ActivationFunctionType
ALU = mybir.AluOpType

B, L, N, D, E = 64, 336, 64, 32, 4
NB = 167          # rfft bins 1..167 (low_freq dropped, nyquist dropped)
B_PER = 8         # batch rows per core
NPAIR = B_PER // 2
NCORES = 8
LCH = 112         # L split into 3 chunks of 112
ND = N * D        # 2048
S2 = 2 * N        # 128 series per pair
KERNELS = (4, 8, 12)
PAD_F, PAD_E = 6, 5
CSLEN = 1 + PAD_F + L + PAD_E   # leading zero + padded series = 348
NOISE_EPS = 0.01
LOSS_COEF = 0.01
NEG_BIG = -1.0e30
JCH = (128, 39)   # NB split for reconstruction lhsT chunks


def _ap_with(a, pattern, extra_offset=0):
    return bass.AP(tensor=a.tensor, offset=a.offset + extra_offset, ap=pattern)


def build_nc():
    nc = bacc.Bacc("TRN2", target_bir_lowering=False, debug=False,
                   enable_asserts=False, num_devices=NCORES)

    x_ext = nc.dram_tensor("x", [B_PER, L, N, D], F32, kind="ExternalInput")
    w2_ext = nc.dram_tensor("w2", [S2, 2], F32, kind="ExternalInput")
    wg_ext = nc.dram_tensor("wg_w", [L, E], F32, kind="ExternalInput")
    wn_ext = nc.dram_tensor("wn_w", [L, E], F32, kind="ExternalInput")
    wgbe_ext = nc.dram_tensor("wg_be", [1, E], F32, kind="ExternalInput")
    wnbe_ext = nc.dram_tensor("wn_be", [1, E], F32, kind="ExternalInput")
    decw_ext = nc.dram_tensor("dec_w", [1, 3], F32, kind="ExternalInput")
    decb_ext = nc.dram_tensor("dec_b", [1, 3], F32, kind="ExternalInput")
    cf_ext = nc.dram_tensor("cf", [L, NB], F32, kind="ExternalInput")
    sfn_ext = nc.dram_tensor("sfn", [L, NB], F32, kind="ExternalInput")
    cr_ext = nc.dram_tensor("cr", [NB, L], BF16, kind="ExternalInput")
    srn_ext = nc.dram_tensor("srn", [NB, L], BF16, kind="ExternalInput")

    out_ext = nc.dram_tensor("out", [B_PER, L, N, D], F32, kind="ExternalOutput")
    stats_ext = nc.dram_tensor("stats", [2, NPAIR * 2 * E], F32, kind="ExternalOutput")

    # row b as [112, 6144]: partition p holds l = 3p..3p+2 (24 KB contiguous)
    x_rows = x_ext.ap().rearrange("b l n d -> b (l n d)")
    out_rows = out_ext.ap().rearrange("b l n d -> b (l n d)")

    with tile.TileContext(nc) as tc, ExitStack() as ctx:
        consts = ctx.enter_context(tc.tile_pool(name="consts", bufs=1))
        rowp = ctx.enter_context(tc.tile_pool(name="rowp", bufs=2))
        x3tp = ctx.enter_context(tc.tile_pool(name="x3tp", bufs=2))
        padp = ctx.enter_context(tc.tile_pool(name="padp", bufs=2))
        map_ = ctx.enter_context(tc.tile_pool(name="map", bufs=2))
        workp = ctx.enter_context(tc.tile_pool(name="workp", bufs=2))
        freqp = ctx.enter_context(tc.tile_pool(name="freqp", bufs=2))
        remtp = ctx.enter_context(tc.tile_pool(name="remtp", bufs=2))
        gtsp = ctx.enter_context(tc.tile_pool(name="gtsp", bufs=2))
        nanop = ctx.enter_context(tc.tile_pool(name="nanop", bufs=2))
        sbp = ctx.enter_context(tc.tile_pool(name="sbp", bufs=2))
        statp = ctx.enter_context(tc.tile_pool(name="statp", bufs=1))
        ps_small = ctx.enter_context(tc.tile_pool(name="ps_small", bufs=2, space="PSUM"))
        ps_dft = ctx.enter_context(tc.tile_pool(name="ps_dft", bufs=2, space="PSUM"))
        ps_seas = ctx.enter_context(tc.tile_pool(name="ps_seas", bufs=2, space="PSUM"))

        def emit_load(p):
            rows = (2 * p, 2 * p + 1)
            rowt = []
            for j in (0, 1):
                rt = rowp.tile([LCH, 3 * ND], F32, tag=f"row{j}")
                nc.sync.dma_start(rt[:], x_rows[rows[j]].rearrange("(p f) -> p f", p=LCH))
                rowt.append(rt)
            return rowt

        # issue the first loads before the constant DMAs so streaming starts
        # immediately
        loaded = {p: emit_load(p) for p in range(min(2, NPAIR))}

        # ---- constants ----
        identity = consts.tile([128, 128], F32, tag="ident")
        make_identity(nc, identity[:])

        cf_t, sfn_t, wg_t, wn_t = [], [], [], []
        for c in range(3):
            l0 = c * LCH
            t1 = consts.tile([LCH, NB], F32, tag=f"cf{c}")
            nc.sync.dma_start(t1[:], cf_ext.ap()[l0:l0 + LCH, :])
            cf_t.append(t1)
            t2 = consts.tile([LCH, NB], F32, tag=f"sfn{c}")
            nc.sync.dma_start(t2[:], sfn_ext.ap()[l0:l0 + LCH, :])
            sfn_t.append(t2)
            t3 = consts.tile([LCH, E], F32, tag=f"wg{c}")
            nc.sync.dma_start(t3[:], wg_ext.ap()[l0:l0 + LCH, :])
            wg_t.append(t3)
            t4 = consts.tile([LCH, E], F32, tag=f"wn{c}")
            nc.sync.dma_start(t4[:], wn_ext.ap()[l0:l0 + LCH, :])
            wn_t.append(t4)

        cr_t, srn_t = [], []
        j0 = 0
        for jc, jlen in enumerate(JCH):
            t1 = consts.tile([jlen, L], BF16, tag=f"cr{jc}")
            nc.sync.dma_start(t1[:], cr_ext.ap()[j0:j0 + jlen, :])
            cr_t.append(t1)
            t2 = consts.tile([jlen, L], BF16, tag=f"srn{jc}")
            nc.sync.dma_start(t2[:], srn_ext.ap()[j0:j0 + jlen, :])
            srn_t.append(t2)
            j0 += jlen

        w2_t = consts.tile([S2, 2], F32, tag="w2")
        nc.sync.dma_start(w2_t[:], w2_ext.ap())

        # [2, E] biases (both partitions hold the same row)
        wgbe_t = consts.tile([2, E], F32, tag="wgbe")
        a = wgbe_ext.ap()
        nc.sync.dma_start(wgbe_t[:], _ap_with(a, [[0, 2], a.ap[1]]))
        wnbe_t = consts.tile([2, E], F32, tag="wnbe")
        a = wnbe_ext.ap()
        nc.sync.dma_start(wnbe_t[:], _ap_with(a, [[0, 2], a.ap[1]]))

        decw_t = consts.tile([S2, 3], F32, tag="decw")
        a = decw_ext.ap()
        nc.sync.dma_start(decw_t[:], _ap_with(a, [[0, S2], a.ap[1]]))
        decb_t = consts.tile([S2, 3], F32, tag="decb")
        a = decb_ext.ap()
        nc.sync.dma_start(decb_t[:], _ap_with(a, [[0, S2], a.ap[1]]))

        stats_sb = statp.tile([2, NPAIR * 2 * E], F32, tag="stats")

        # ---- per-pair pipeline ----
        for p in range(NPAIR):
            rows = (2 * p, 2 * p + 1)
            rowt = loaded.pop(p) if p in loaded else emit_load(p)

            # x3T extraction: x3t[i][p, j*64+n] = rowt[j][p, i*2048 + n*32]
            # (chunk i partition p holds series values at l = 3p + i)
            x3t = []
            eng = [nc.gpsimd.tensor_copy, nc.vector.tensor_copy, nc.scalar.copy]
            for i in range(3):
                t = x3tp.tile([LCH, S2], F32, tag=f"x3t{i}")
                for j in (0, 1):
                    src = rowt[j][:]
                    eng[i](t[:, j * N:(j + 1) * N],
                           _ap_with(src, [src.ap[0], [D, N]],
                                    extra_offset=i * ND))
                x3t.append(t)

            for j in (0, 1):
                dst = out_rows[rows[j]].rearrange("(p f) -> p f", p=LCH)
                if j == 0:
                    nc.gpsimd.dma_start(dst, rowt[j][:])
                else:
                    nc.scalar.dma_start(dst, rowt[j][:])

            # padbuf: [zero | 6 x front pad | x3 | 5 x end pad]; cumsum cs
            padbuf = padp.tile([S2, CSLEN], F32, tag="padbuf")
            nc.vector.memset(padbuf[:, 0:1], 0.0)
            pb = padbuf[:]
            cpeng = [nc.scalar.copy, nc.vector.tensor_copy, nc.scalar.copy]
            for i in range(3):
                pt = ps_small.tile([S2, LCH], F32, tag="ps_sm")
                nc.tensor.transpose(pt[:], x3t[i][:], identity[0:LCH, 0:LCH])
                cpeng[i](_ap_with(pb, [pb.ap[0], [3, LCH]],
                                  extra_offset=1 + PAD_F + i),
                         pt[:])
            x3a = padbuf[:, 1 + PAD_F:1 + PAD_F + L]
            nc.scalar.copy(padbuf[:, 1:1 + PAD_F],
                           padbuf[:, 1 + PAD_F:2 + PAD_F].to_broadcast([S2, PAD_F]))
            nc.scalar.copy(padbuf[:, 1 + PAD_F + L:CSLEN],
                           padbuf[:, PAD_F + L:1 + PAD_F + L].to_broadcast([S2, PAD_E]))
            cs = padp.tile([S2, CSLEN], F32, tag="cs")
            nc.vector.tensor_tensor_scan(cs[:], padbuf[:], padbuf[:], 0.0,
                                         op0=ALU.add, op1=ALU.bypass)

            # decomp softmax mix; ma_k = (cs[p0+k]-cs[p0])/k, p0 = l+6-front_k
            eds = []
            for d in range(3):
                ed = map_.tile([S2, L], F32, tag=f"ed{d}")
                nc.scalar.activation(ed[:], x3a, AF.Exp,
                                     bias=decb_t[:, d:d + 1], scale=decw_t[:, d:d + 1])
                eds.append(ed)
            den = workp.tile([S2, L], F32, tag="den")
            nc.gpsimd.tensor_add(den[:], eds[0][:], eds[1][:])
            nc.gpsimd.tensor_add(den[:], den[:], eds[2][:])
            recip = workp.tile([S2, L], F32, tag="recip")
            nc.vector.reciprocal(recip[:], den[:])

            num = workp.tile([S2, L], F32, tag="num")
            tmp = workp.tile([S2, L], F32, tag="tmpnum")
            for d, k in enumerate(KERNELS):
                front = (k - 1) - (k - 1) // 2
                p0 = PAD_F - front
                wsum = workp.tile([S2, L], F32, tag=f"wsum{d}")
                weng = nc.gpsimd if d < 2 else nc.vector
                weng.tensor_sub(wsum[:], cs[:, p0 + k:p0 + k + L],
                                cs[:, p0:p0 + L])
                dst = num if d == 0 else tmp
                nc.vector.scalar_tensor_tensor(
                    out=dst[:], in0=eds[d][:], scalar=1.0 / k, in1=wsum[:],
                    op0=ALU.mult, op1=ALU.mult)
                if d > 0:
                    nc.vector.tensor_add(num[:], num[:], tmp[:])
            trend = workp.tile([S2, L], F32, tag="trend")
            nc.vector.tensor_mul(trend[:], num[:], recip[:])

            # forward DFT: RE/IM [128, 167]
            re_ps = ps_dft.tile([S2, NB], F32, tag="ps_re")
            im_ps = ps_dft.tile([S2, NB], F32, tag="ps_im")
            for c in range(3):
                nc.tensor.matmul(re_ps[:], x3t[c][:], cf_t[c][:],
                                 start=(c == 0), stop=(c == 2))
            for c in range(3):
                nc.tensor.matmul(im_ps[:], x3t[c][:], sfn_t[c][:],
                                 start=(c == 0), stop=(c == 2))

            # amp^2, top-3 mask, masked spectra
            sqre = freqp.tile([S2, NB], F32, tag="sqre")
            nc.scalar.activation(sqre[:], re_ps[:], AF.Square)
            sqim = freqp.tile([S2, NB], F32, tag="sqim")
            nc.scalar.activation(sqim[:], im_ps[:], AF.Square)
            amp2 = freqp.tile([S2, NB], F32, tag="amp2")
            nc.gpsimd.tensor_add(amp2[:], sqre[:], sqim[:])
            m8 = freqp.tile([S2, 8], F32, tag="m8")
            nc.vector.max(out=m8[:], in_=amp2[:])
            mask = freqp.tile([S2, NB], F32, tag="mask")
            nc.vector.tensor_scalar(mask[:], amp2[:], m8[:, 2:3], None, op0=ALU.is_ge)
            rem = freqp.tile([S2, NB], F32, tag="rem")
            nc.vector.tensor_mul(rem[:], re_ps[:], mask[:])
            imm = freqp.tile([S2, NB], F32, tag="imm")
            nc.vector.tensor_mul(imm[:], im_ps[:], mask[:])

            # transpose masked spectra to [j, s]
            remt, immt = [], []
            j0 = 0
            for jc, jlen in enumerate(JCH):
                pt = ps_small.tile([jlen, S2], F32, tag="ps_sm")
                nc.tensor.transpose(pt[:], rem[:, j0:j0 + jlen], identity[0:S2, 0:S2])
                t = remtp.tile([jlen, S2], BF16, tag=f"remt{jc}")
                nc.scalar.copy(t[:], pt[:])
                remt.append(t)
                pt2 = ps_small.tile([jlen, S2], F32, tag="ps_sm")
                nc.tensor.transpose(pt2[:], imm[:, j0:j0 + jlen], identity[0:S2, 0:S2])
                t2 = remtp.tile([jlen, S2], BF16, tag=f"immt{jc}")
                nc.scalar.copy(t2[:], pt2[:])
                immt.append(t2)
                j0 += jlen

            # reconstruction
            seas_ps = ps_seas.tile([S2, L], F32, tag="ps_seas")
            nc.tensor.matmul(seas_ps[:], remt[0][:], cr_t[0][:], start=True, stop=False)
            nc.tensor.matmul(seas_ps[:], remt[1][:], cr_t[1][:], start=False, stop=False)
            nc.tensor.matmul(seas_ps[:], immt[0][:], srn_t[0][:], start=False, stop=False)
            nc.tensor.matmul(seas_ps[:], immt[1][:], srn_t[1][:], start=False, stop=True)

            # new_x = x3 + seas + trend
            newx0 = workp.tile([S2, L], F32, tag="newx0")
            nc.vector.tensor_add(newx0[:], seas_ps[:], trend[:])
            newx = workp.tile([S2, L], F32, tag="newx")
            nc.gpsimd.tensor_add(newx[:], newx0[:], x3a)

            # gT chunks (block-diagonal start_w -> per-row g), gating GEMMs
            gts = []
            for c in range(3):
                pt = ps_small.tile([LCH, 2], F32, tag="ps_sm")
                nc.tensor.matmul(pt[:], newx[:, c * LCH:(c + 1) * LCH], w2_t[:],
                                 start=True, stop=True)
                t = gtsp.tile([LCH, 2], F32, tag=f"gts{c}")
                nc.scalar.copy(t[:], pt[:])
                gts.append(t)
            clean_ps = ps_small.tile([2, E], F32, tag="ps_sm")
            for c in range(3):
                nc.tensor.matmul(clean_ps[:], gts[c][:], wg_t[c][:],
                                 start=(c == 0), stop=(c == 2))
            wn_ps = ps_small.tile([2, E], F32, tag="ps_sm")
            for c in range(3):
                nc.tensor.matmul(wn_ps[:], gts[c][:], wn_t[c][:],
                                 start=(c == 0), stop=(c == 2))

            # nano stage [2, E]: stats + device gate-sum s (std == 1)
            clean = stats_sb[0:2, p * 2 * E:p * 2 * E + E]
            nc.vector.tensor_add(clean, clean_ps[:], wgbe_t[:])
            wnpre = stats_sb[0:2, p * 2 * E + E:p * 2 * E + 2 * E]
            nc.vector.tensor_add(wnpre, wn_ps[:], wnbe_t[:])
            nc.sync.dma_start(stats_ext.ap()[:, p * 2 * E:(p + 1) * 2 * E],
                              stats_sb[0:2, p * 2 * E:(p + 1) * 2 * E])



    nc.compile()
    return nc


def _dft_consts():
    j = np.arange(1, NB + 1, dtype=np.float64)
    t = np.arange(L, dtype=np.float64)
    ang = 2.0 * np.pi * np.outer(t, j) / L          # [L, NB]
    cf = np.cos(ang).astype(np.float32)
    sfn = (-np.sin(ang)).astype(np.float32)
    cr = ((2.0 / L) * np.cos(ang.T)).astype(ml_dtypes.bfloat16)   # [NB, L]
    srn = (-(2.0 / L) * np.sin(ang.T)).astype(ml_dtypes.bfloat16)
    return cf, sfn, cr, srn


_CACHE = {}


PERM = np.array([3 * p + i for i in range(3) for p in range(LCH)])


def make_in_maps(x, noise, start_w, start_b, wg_w, wg_b, wn_w, wn_b,
                 decomp_w, decomp_b):
    cf, sfn, cr, srn = _dft_consts()
    cf = np.ascontiguousarray(cf[PERM])
    sfn = np.ascontiguousarray(sfn[PERM])
    f32 = lambda v: np.ascontiguousarray(v, dtype=np.float32)
    w2 = np.zeros((S2, 2), np.float32)
    w2[0:N, 0] = np.asarray(start_w, np.float32)[:, 0]
    w2[N:S2, 1] = np.asarray(start_w, np.float32)[:, 0]
    wg_be = f32(wg_b + start_b[0] * wg_w.sum(0))[None, :]
    wn_be = f32(wn_b + start_b[0] * wn_w.sum(0))[None, :]
    common = {
        "w2": w2,
        "wg_w": f32(wg_w),
        "wn_w": f32(wn_w),
        "wg_be": wg_be,
        "wn_be": wn_be,
        "dec_w": f32(decomp_w).reshape(1, 3),
        "dec_b": f32(decomp_b).reshape(1, 3),
        "cf": cf, "sfn": sfn, "cr": cr, "srn": srn,
    }
    in_maps = []
    for i in range(NCORES):
        m = dict(common)
        m["x"] = f32(x[i * B_PER:(i + 1) * B_PER])
        in_maps.append(m)
    return in_maps


def unpack_stats(stats_list):
    """stats [2, NPAIR*8] per core -> clean [B, E], wnpre [B, E]."""
    clean = np.zeros((B, E), np.float32)
    wnpre = np.zeros((B, E), np.float32)
    for i, st in enumerate(stats_list):
        st = st.reshape(2, NPAIR, 2 * E)
        for p in range(NPAIR):
            for j in (0, 1):
                r = i * B_PER + 2 * p + j
                clean[r] = st[j, p, :E]
                wnpre[r] = st[j, p, E:]
    return clean, wnpre


def host_tail(stats_list, noise):
    clean32, wnpre32 = unpack_stats(stats_list)
    clean = clean32.astype(np.float64)
    std = np.log1p(np.exp(wnpre32.astype(np.float64))) + NOISE_EPS
    noisy = clean + noise.astype(np.float64) * std

    order = np.argsort(-noisy, axis=1, kind="stable")
    v1 = np.take_along_axis(noisy, order[:, 0:1], 1)
    v2 = np.take_along_axis(noisy, order[:, 1:2], 1)
    v3 = np.take_along_axis(noisy, order[:, 2:3], 1)
    e2 = np.exp(v2 - v1)
    den = 1.0 + e2
    gates = np.zeros((B, E), np.float64)
    np.put_along_axis(gates, order[:, 0:1], 1.0 / den, 1)
    np.put_along_axis(gates, order[:, 1:2], e2 / den, 1)
    imp = gates.sum(0)

    nerf = np.vectorize(math.erf)
    phi = lambda z: 0.5 * (1.0 + nerf(z / math.sqrt(2.0)))
    prob = np.where(noisy > v3, phi((clean - v3) / std), phi((clean - v2) / std))
    load = prob.sum(0)

    def cv2(v):
        return v.var(ddof=1) / (v.mean() ** 2 + 1e-10)

    return np.float32(LOSS_COEF * (cv2(imp) + cv2(load)))


def kernel(**inputs):
    if "nc" not in _CACHE:
        _CACHE["nc"] = build_nc()
    nc = _CACHE["nc"]

    inputs = {k: np.asarray(v) for k, v in inputs.items()}
    inputs.pop("padding_mask", None)
    in_maps = make_in_maps(**inputs)

    res = run_bass_kernel_spmd(nc, in_maps, core_ids=list(range(NCORES)))

    out = np.empty((B, L, N, D), np.float32)
    stats_list = []
    for i in range(NCORES):
        out[i * B_PER:(i + 1) * B_PER] = res.results[i]["out"].reshape(
            B_PER, L, N, D)
        stats_list.append(res.results[i]["stats"])
    loss = host_tail(stats_list, inputs["noise"])
    return out, loss


# revision 21
# speedup vs baseline: 1.1146x; 1.1146x over previous
"""Trainium2 Bass kernel for nn_AMS_18975165514201 (moe_routing).

Data-parallel over batch B=64 on 8 NeuronCores (8 rows/core), no
collectives.  Rows are processed in PAIRS so the series dimension packs
the full 128 partitions (2 rows x 64 channels).  Per pair, on device:
  - stream both rows of x (2 x 2.75 MB) into SBUF
  - extract x3 = x[..., 0] with strided on-chip copies -> [112, 128] x3T
  - series_decomp_multi: one cumsum (tensor_tensor_scan) over an
    edge-padded tile, window sums by shifted subtraction, mixed by a
    softmax of decomp projections
  - Fourier layer: DFT via matmuls against cos/sin constants, per-series
    top-3 bin selection (InstMax + is_ge mask), masked reconstruction
  - gating GEMMs (block-diagonal start_w packs both rows) -> per-row
    top-2 softmax gate-sum s
  - scale the SBUF-resident rows by s and stream them out
The gate-sum s equals 1 to within 1e-7 for ANY noise_std (softmax rows
sum to 1), so the device uses std=1 for its own top-2 pick; "stats"
carries clean logits + wnpre per row and the host combine computes the
exact noise_std/importance/load/balance loss in float64.
"""

import math
from contextlib import ExitStack

import ml_dtypes
import numpy as np

import concourse.bass as bass
import concourse.tile as tile
from concourse import bacc, mybir
from concourse.bass_utils import run_bass_kernel_spmd
from concourse.masks import make_identity

F32 = mybir.dt.float32
BF16 = mybir.dt.bfloat16
AX = mybir.AxisListType
AF = mybir.ActivationFunctionType
ALU = mybir.AluOpType

B, L, N, D, E = 64, 336, 64, 32, 4
NB = 167          # rfft bins 1..167 (low_freq dropped, nyquist dropped)
B_PER = 8         # batch rows per core
NPAIR = B_PER // 2
NCORES = 8
LCH = 112         # L split into 3 chunks of 112
ND = N * D        # 2048
S2 = 2 * N        # 128 series per pair
KERNELS = (4, 8, 12)
PAD_F, PAD_E = 6, 5
CSLEN = 1 + PAD_F + L + PAD_E   # leading zero + padded series = 348
NOISE_EPS = 0.01
LOSS_COEF = 0.01
NEG_BIG = -1.0e30
JCH = (128, 39)   # NB split for reconstruction lhsT chunks


def _ap_with(a, pattern, extra_offset=0):
    return bass.AP(tensor=a.tensor, offset=a.offset + extra_offset, ap=pattern)


def build_nc():
    nc = bacc.Bacc("TRN2", target_bir_lowering=False, debug=False,
                   enable_asserts=False, num_devices=NCORES)

    x_ext = nc.dram_tensor("x", [B_PER, L, N, D], F32, kind="ExternalInput")
    w2_ext = nc.dram_tensor("w2", [S2, 2], F32, kind="ExternalInput")
    wg_ext = nc.dram_tensor("wg_w", [L, E], F32, kind="ExternalInput")
    wn_ext = nc.dram_tensor("wn_w", [L, E], F32, kind="ExternalInput")
    wgbe_ext = nc.dram_tensor("wg_be", [1, E], F32, kind="ExternalInput")
    wnbe_ext = nc.dram_tensor("wn_be", [1, E], F32, kind="ExternalInput")
    decw_ext = nc.dram_tensor("dec_w", [1, 3], F32, kind="ExternalInput")
    decb_ext = nc.dram_tensor("dec_b", [1, 3], F32, kind="ExternalInput")
    cf_ext = nc.dram_tensor("cf", [L, NB], F32, kind="ExternalInput")
    sfn_ext = nc.dram_tensor("sfn", [L, NB], F32, kind="ExternalInput")
    cr_ext = nc.dram_tensor("cr", [NB, L], BF16, kind="ExternalInput")
    srn_ext = nc.dram_tensor("srn", [NB, L], BF16, kind="ExternalInput")

    out_ext = nc.dram_tensor("out", [B_PER, L, N, D], F32, kind="ExternalOutput")
    stats_ext = nc.dram_tensor("stats", [2, NPAIR * 2 * E], F32, kind="ExternalOutput")

    # row b as [112, 6144]: partition p holds l = 3p..3p+2 (24 KB contiguous)
    x_rows = x_ext.ap().rearrange("b l n d -> b (l n d)")
    out_rows = out_ext.ap().rearrange("b l n d -> b (l n d)")

    with tile.TileContext(nc) as tc, ExitStack() as ctx:
        consts = ctx.enter_context(tc.tile_pool(name="consts", bufs=1))
        rowp = ctx.enter_context(tc.tile_pool(name="rowp", bufs=3))
        x3tp = ctx.enter_context(tc.tile_pool(name="x3tp", bufs=2))
        padp = ctx.enter_context(tc.tile_pool(name="padp", bufs=2))
        map_ = ctx.enter_context(tc.tile_pool(name="map", bufs=2))
        workp = ctx.enter_context(tc.tile_pool(name="workp", bufs=1))
        freqp = ctx.enter_context(tc.tile_pool(name="freqp", bufs=1))
        remtp = ctx.enter_context(tc.tile_pool(name="remtp", bufs=2))
        gtsp = ctx.enter_context(tc.tile_pool(name="gtsp", bufs=2))
        nanop = ctx.enter_context(tc.tile_pool(name="nanop", bufs=2))
        sbp = ctx.enter_context(tc.tile_pool(name="sbp", bufs=2))
        statp = ctx.enter_context(tc.tile_pool(name="statp", bufs=1))
        ps_small = ctx.enter_context(tc.tile_pool(name="ps_small", bufs=2, space="PSUM"))
        ps_dft = ctx.enter_context(tc.tile_pool(name="ps_dft", bufs=2, space="PSUM"))
        ps_seas = ctx.enter_context(tc.tile_pool(name="ps_seas", bufs=2, space="PSUM"))

        def emit_load(p):
            rows = (2 * p, 2 * p + 1)
            rowt = []
            for j in (0, 1):
                rt = rowp.tile([LCH, 3 * ND], F32, tag=f"row{j}")
                nc.sync.dma_start(rt[:], x_rows[rows[j]].rearrange("(p f) -> p f", p=LCH))
                rowt.append(rt)
            return rowt

        # issue the first loads before the constant DMAs so streaming starts
        # immediately
        loaded = {p: emit_load(p) for p in range(min(3, NPAIR))}

        # ---- constants ----
        identity = consts.tile([128, 128], F32, tag="ident")
        make_identity(nc, identity[:])

        cf_t, sfn_t, wg_t, wn_t = [], [], [], []
        for c in range(3):
            l0 = c * LCH
            t1 = consts.tile([LCH, NB], F32, tag=f"cf{c}")
            nc.sync.dma_start(t1[:], cf_ext.ap()[l0:l0 + LCH, :])
            cf_t.append(t1)
            t2 = consts.tile([LCH, NB], F32, tag=f"sfn{c}")
            nc.sync.dma_start(t2[:], sfn_ext.ap()[l0:l0 + LCH, :])
            sfn_t.append(t2)
            t3 = consts.tile([LCH, E], F32, tag=f"wg{c}")
            nc.sync.dma_start(t3[:], wg_ext.ap()[l0:l0 + LCH, :])
            wg_t.append(t3)
            t4 = consts.tile([LCH, E], F32, tag=f"wn{c}")
            nc.sync.dma_start(t4[:], wn_ext.ap()[l0:l0 + LCH, :])
            wn_t.append(t4)

        cr_t, srn_t = [], []
        j0 = 0
        for jc, jlen in enumerate(JCH):
            t1 = consts.tile([jlen, L], BF16, tag=f"cr{jc}")
            nc.sync.dma_start(t1[:], cr_ext.ap()[j0:j0 + jlen, :])
            cr_t.append(t1)
            t2 = consts.tile([jlen, L], BF16, tag=f"srn{jc}")
            nc.sync.dma_start(t2[:], srn_ext.ap()[j0:j0 + jlen, :])
            srn_t.append(t2)
            j0 += jlen

        w2_t = consts.tile([S2, 2], F32, tag="w2")
        nc.sync.dma_start(w2_t[:], w2_ext.ap())

        # [2, E] biases (both partitions hold the same row)
        wgbe_t = consts.tile([2, E], F32, tag="wgbe")
        a = wgbe_ext.ap()
        nc.sync.dma_start(wgbe_t[:], _ap_with(a, [[0, 2], a.ap[1]]))
        wnbe_t = consts.tile([2, E], F32, tag="wnbe")
        a = wnbe_ext.ap()
        nc.sync.dma_start(wnbe_t[:], _ap_with(a, [[0, 2], a.ap[1]]))

        decw_t = consts.tile([S2, 3], F32, tag="decw")
        a = decw_ext.ap()
        nc.sync.dma_start(decw_t[:], _ap_with(a, [[0, S2], a.ap[1]]))
        decb_t = consts.tile([S2, 3], F32, tag="decb")
        a = decb_ext.ap()
        nc.sync.dma_start(decb_t[:], _ap_with(a, [[0, S2], a.ap[1]]))

        stats_sb = statp.tile([2, NPAIR * 2 * E], F32, tag="stats")

        # ---- per-pair pipeline ----
        for p in range(NPAIR):
            rows = (2 * p, 2 * p + 1)
            rowt = loaded.pop(p) if p in loaded else emit_load(p)

            # x3T extraction: x3t[i][p, j*64+n] = rowt[j][p, i*2048 + n*32]
            # (chunk i partition p holds series values at l = 3p + i)
            x3t = []
            eng = [nc.gpsimd.tensor_copy, nc.gpsimd.tensor_copy, nc.gpsimd.tensor_copy]
            for i in range(3):
                t = x3tp.tile([LCH, S2], F32, tag=f"x3t{i}")
                for j in (0, 1):
                    src = rowt[j][:]
                    eng[i](t[:, j * N:(j + 1) * N],
                           _ap_with(src, [src.ap[0], [D, N]],
                                    extra_offset=i * ND))
                x3t.append(t)

            for j in (0, 1):
                dst = out_rows[rows[j]].rearrange("(p f) -> p f", p=LCH)
                if j == 0:
                    nc.gpsimd.dma_start(dst, rowt[j][:])
                else:
                    nc.scalar.dma_start(dst, rowt[j][:])

            # padbuf: [zero | 6 x front pad | x3 | 5 x end pad]; cumsum cs
            padbuf = padp.tile([S2, CSLEN], F32, tag="padbuf")
            nc.vector.memset(padbuf[:, 0:1], 0.0)
            pb = padbuf[:]
            cpeng = [nc.scalar.copy, nc.vector.tensor_copy, nc.scalar.copy]
            for i in range(3):
                pt = ps_small.tile([S2, LCH], F32, tag="ps_sm")
                nc.tensor.transpose(pt[:], x3t[i][:], identity[0:LCH, 0:LCH])
                cpeng[i](_ap_with(pb, [pb.ap[0], [3, LCH]],
                                  extra_offset=1 + PAD_F + i),
                         pt[:])
            x3a = padbuf[:, 1 + PAD_F:1 + PAD_F + L]
            nc.scalar.copy(padbuf[:, 1:1 + PAD_F],
                           padbuf[:, 1 + PAD_F:2 + PAD_F].to_broadcast([S2, PAD_F]))
            nc.scalar.copy(padbuf[:, 1 + PAD_F + L:CSLEN],
                           padbuf[:, PAD_F + L:1 + PAD_F + L].to_broadcast([S2, PAD_E]))
            cs = padp.tile([S2, CSLEN], F32, tag="cs")
            nc.vector.tensor_tensor_scan(cs[:], padbuf[:], padbuf[:], 0.0,
                                         op0=ALU.add, op1=ALU.bypass)

            # decomp softmax mix; ma_k = (cs[p0+k]-cs[p0])/k, p0 = l+6-front_k
            eds = []
            for d in range(3):
                ed = map_.tile([S2, L], F32, tag=f"ed{d}")
                nc.scalar.activation(ed[:], x3a, AF.Exp,
                                     bias=decb_t[:, d:d + 1], scale=decw_t[:, d:d + 1])
                eds.append(ed)
            den = workp.tile([S2, L], F32, tag="den")
            nc.vector.tensor_add(den[:], eds[0][:], eds[1][:])
            nc.vector.tensor_add(den[:], den[:], eds[2][:])
            recip = workp.tile([S2, L], F32, tag="recip")
            nc.vector.reciprocal_approx_fast(recip[:], den[:])

            num = workp.tile([S2, L], F32, tag="num")
            tmp = workp.tile([S2, L], F32, tag="tmpnum")
            for d, k in enumerate(KERNELS):
                front = (k - 1) - (k - 1) // 2
                p0 = PAD_F - front
                wsum = workp.tile([S2, L], F32, tag=f"wsum{d}")
                nc.vector.tensor_sub(wsum[:], cs[:, p0 + k:p0 + k + L],
                                     cs[:, p0:p0 + L])
                dst = num if d == 0 else tmp
                nc.vector.scalar_tensor_tensor(
                    out=dst[:], in0=eds[d][:], scalar=1.0 / k, in1=wsum[:],
                    op0=ALU.mult, op1=ALU.mult)
                if d > 0:
                    nc.vector.tensor_add(num[:], num[:], tmp[:])
            trend = workp.tile([S2, L], F32, tag="trend")
            nc.vector.tensor_mul(trend[:], num[:], recip[:])

            # forward DFT: RE/IM [128, 167]
            re_ps = ps_dft.tile([S2, NB], F32, tag="ps_re")
            im_ps = ps_dft.tile([S2, NB], F32, tag="ps_im")
            for c in range(3):
                nc.tensor.matmul(re_ps[:], x3t[c][:], cf_t[c][:],
                                 start=(c == 0), stop=(c == 2))
            for c in range(3):
                nc.tensor.matmul(im_ps[:], x3t[c][:], sfn_t[c][:],
                                 start=(c == 0), stop=(c == 2))

            # amp^2, top-3 mask, masked spectra
            sqre = freqp.tile([S2, NB], F32, tag="sqre")
            nc.scalar.activation(sqre[:], re_ps[:], AF.Square)
            sqim = freqp.tile([S2, NB], F32, tag="sqim")
            nc.scalar.activation(sqim[:], im_ps[:], AF.Square)
            amp2 = freqp.tile([S2, NB], F32, tag="amp2")
            nc.vector.tensor_add(amp2[:], sqre[:], sqim[:])
            m8 = freqp.tile([S2, 8], F32, tag="m8")
            nc.vector.max(out=m8[:], in_=amp2[:])
            mask = freqp.tile([S2, NB], F32, tag="mask")
            nc.vector.tensor_scalar(mask[:], amp2[:], m8[:, 2:3], None, op0=ALU.is_ge)
            rem = freqp.tile([S2, NB], F32, tag="rem")
            nc.vector.tensor_mul(rem[:], re_ps[:], mask[:])
            imm = freqp.tile([S2, NB], F32, tag="imm")
            nc.vector.tensor_mul(imm[:], im_ps[:], mask[:])

            # transpose masked spectra to [j, s]
            remt, immt = [], []
            j0 = 0
            for jc, jlen in enumerate(JCH):
                pt = ps_small.tile([jlen, S2], F32, tag="ps_sm")
                nc.tensor.transpose(pt[:], rem[:, j0:j0 + jlen], identity[0:S2, 0:S2])
                t = remtp.tile([jlen, S2], BF16, tag=f"remt{jc}")
                nc.scalar.copy(t[:], pt[:])
                remt.append(t)
                pt2 = ps_small.tile([jlen, S2], F32, tag="ps_sm")
                nc.tensor.transpose(pt2[:], imm[:, j0:j0 + jlen], identity[0:S2, 0:S2])
                t2 = remtp.tile([jlen, S2], BF16, tag=f"immt{jc}")
                nc.scalar.copy(t2[:], pt2[:])
                immt.append(t2)
                j0 += jlen

            # reconstruction
            seas_ps = ps_seas.tile([S2, L], F32, tag="ps_seas")
            nc.tensor.matmul(seas_ps[:], remt[0][:], cr_t[0][:], start=True, stop=False)
            nc.tensor.matmul(seas_ps[:], remt[1][:], cr_t[1][:], start=False, stop=False)
            nc.tensor.matmul(seas_ps[:], immt[0][:], srn_t[0][:], start=False, stop=False)
            nc.tensor.matmul(seas_ps[:], immt[1][:], srn_t[1][:], start=False, stop=True)

            # new_x = x3 + seas + trend
            newx0 = workp.tile([S2, L], F32, tag="newx0")
            nc.vector.tensor_add(newx0[:], seas_ps[:], trend[:])
            newx = workp.tile([S2, L], F32, tag="newx")
            nc.vector.tensor_add(newx[:], newx0[:], x3a)

            # gT chunks (block-diagonal start_w -> per-row g), gating GEMMs
            gts = []
            for c in range(3):
                pt = ps_small.tile([LCH, 2], F32, tag="ps_sm")
                nc.tensor.matmul(pt[:], newx[:, c * LCH:(c + 1) * LCH], w2_t[:],
                                 start=True, stop=True)
                t = gtsp.tile([LCH, 2], F32, tag=f"gts{c}")
                nc.scalar.copy(t[:], pt[:])
                gts.append(t)
            clean_ps = ps_small.tile([2, E], F32, tag="ps_sm")
            for c in range(3):
                nc.tensor.matmul(clean_ps[:], gts[c][:], wg_t[c][:],
                                 start=(c == 0), stop=(c == 2))
            wn_ps = ps_small.tile([2, E], F32, tag="ps_sm")
            for c in range(3):
                nc.tensor.matmul(wn_ps[:], gts[c][:], wn_t[c][:],
                                 start=(c == 0), stop=(c == 2))

            # nano stage [2, E]: stats + device gate-sum s (std == 1)
            clean = stats_sb[0:2, p * 2 * E:p * 2 * E + E]
            nc.vector.tensor_add(clean, clean_ps[:], wgbe_t[:])
            wnpre = stats_sb[0:2, p * 2 * E + E:p * 2 * E + 2 * E]
            nc.vector.tensor_add(wnpre, wn_ps[:], wnbe_t[:])
            nc.sync.dma_start(stats_ext.ap()[:, p * 2 * E:(p + 1) * 2 * E],
                              stats_sb[0:2, p * 2 * E:(p + 1) * 2 * E])



    nc.compile()
    return nc


def _dft_consts():
    j = np.arange(1, NB + 1, dtype=np.float64)
    t = np.arange(L, dtype=np.float64)
    ang = 2.0 * np.pi * np.outer(t, j) / L          # [L, NB]
    cf = np.cos(ang).astype(np.float32)
    sfn = (-np.sin(ang)).astype(np.float32)
    cr = ((2.0 / L) * np.cos(ang.T)).astype(ml_dtypes.bfloat16)   # [NB, L]
    srn = (-(2.0 / L) * np.sin(ang.T)).astype(ml_dtypes.bfloat16)
    return cf, sfn, cr, srn


_CACHE = {}


PERM = np.array([3 * p + i for i in range(3) for p in range(LCH)])


def make_in_maps(x, noise, start_w, start_b, wg_w, wg_b, wn_w, wn_b,
                 decomp_w, decomp_b):
    cf, sfn, cr, srn = _dft_consts()
    cf = np.ascontiguousarray(cf[PERM])
    sfn = np.ascontiguousarray(sfn[PERM])
    f32 = lambda v: np.ascontiguousarray(v, dtype=np.float32)
    w2 = np.zeros((S2, 2), np.float32)
    w2[0:N, 0] = np.asarray(start_w, np.float32)[:, 0]
    w2[N:S2, 1] = np.asarray(start_w, np.float32)[:, 0]
    wg_be = f32(wg_b + start_b[0] * wg_w.sum(0))[None, :]
    wn_be = f32(wn_b + start_b[0] * wn_w.sum(0))[None, :]
    common = {
        "w2": w2,
        "wg_w": f32(wg_w),
        "wn_w": f32(wn_w),
        "wg_be": wg_be,
        "wn_be": wn_be,
        "dec_w": f32(decomp_w).reshape(1, 3),
        "dec_b": f32(decomp_b).reshape(1, 3),
        "cf": cf, "sfn": sfn, "cr": cr, "srn": srn,
    }
    in_maps = []
    for i in range(NCORES):
        m = dict(common)
        m["x"] = f32(x[i * B_PER:(i + 1) * B_PER])
        in_maps.append(m)
    return in_maps


def unpack_stats(stats_list):
    """stats [2, NPAIR*8] per core -> clean [B, E], wnpre [B, E]."""
    clean = np.zeros((B, E), np.float32)
    wnpre = np.zeros((B, E), np.float32)
    for i, st in enumerate(stats_list):
        st = st.reshape(2, NPAIR, 2 * E)
        for p in range(NPAIR):
            for j in (0, 1):
                r = i * B_PER + 2 * p + j
                clean[r] = st[j, p, :E]
                wnpre[r] = st[j, p, E:]
    return clean, wnpre


def host_tail(stats_list, noise):
    clean32, wnpre32 = unpack_stats(stats_list)
    clean = clean32.astype(np.float64)
    std = np.log1p(np.exp(wnpre32.astype(np.float64))) + NOISE_EPS
    noisy = clean + noise.astype(np.float64) * std

    order = np.argsort(-noisy, axis=1, kind="stable")
    v1 = np.take_along_axis(noisy, order[:, 0:1], 1)
    v2 = np.take_along_axis(noisy, order[:, 1:2], 1)
    v3 = np.take_along_axis(noisy, order[:, 2:3], 1)
    e2 = np.exp(v2 - v1)
    den = 1.0 + e2
    gates = np.zeros((B, E), np.float64)
    np.put_along_axis(gates, order[:, 0:1], 1.0 / den, 1)
    np.put_along_axis(gates, order[:, 1:2], e2 / den, 1)
    imp = gates.sum(0)

    nerf = np.vectorize(math.erf)
    phi = lambda z: 0.5 * (1.0 + nerf(z / math.sqrt(2.0)))
    prob = np.where(noisy > v3, phi((clean - v3) / std), phi((clean - v2) / std))
    load = prob.sum(0)

    def cv2(v):
        return v.var(ddof=1) / (v.mean() ** 2 + 1e-10)

    return np.float32(LOSS_COEF * (cv2(imp) + cv2(load)))


def kernel(**inputs):
    if "nc" not in _CACHE:
        _CACHE["nc"] = build_nc()
    nc = _CACHE["nc"]

    inputs = {k: np.asarray(v) for k, v in inputs.items()}
    inputs.pop("padding_mask", None)
    in_maps = make_in_maps(**inputs)

    res = run_bass_kernel_spmd(nc, in_maps, core_ids=list(range(NCORES)))

    out = np.empty((B, L, N, D), np.float32)
    stats_list = []
    for i in range(NCORES):
        out[i * B_PER:(i + 1) * B_PER] = res.results[i]["out"].reshape(
            B_PER, L, N, D)
        stats_list.append(res.results[i]["stats"])
    loss = host_tail(stats_list, inputs["noise"])
    return out, loss


# revision 24
# speedup vs baseline: 1.1691x; 1.0489x over previous
"""Trainium2 Bass kernel for nn_AMS_18975165514201 (moe_routing).

Data-parallel over batch B=64 on 8 NeuronCores (8 rows/core), no
collectives.  Rows are processed in PAIRS so the series dimension packs
the full 128 partitions (2 rows x 64 channels).  Per pair, on device:
  - stream both rows of x (2 x 2.75 MB) into SBUF
  - extract x3 = x[..., 0] with strided on-chip copies -> [112, 128] x3T
  - series_decomp_multi: one cumsum (tensor_tensor_scan) over an
    edge-padded tile, window sums by shifted subtraction, mixed by a
    softmax of decomp projections
  - Fourier layer: DFT via matmuls against cos/sin constants, per-series
    top-3 bin selection (InstMax + is_ge mask), masked reconstruction
  - gating GEMMs (block-diagonal start_w packs both rows) -> per-row
    top-2 softmax gate-sum s
  - scale the SBUF-resident rows by s and stream them out
The gate-sum s equals 1 to within 1e-7 for ANY noise_std (softmax rows
sum to 1), so the device uses std=1 for its own top-2 pick; "stats"
carries clean logits + wnpre per row and the host combine computes the
exact noise_std/importance/load/balance loss in float64.
"""

import math
from contextlib import ExitStack

import ml_dtypes
import numpy as np

import concourse.bass as bass
import concourse.tile as tile
from concourse import bacc, mybir
from concourse.bass_utils import run_bass_kernel_spmd
from concourse.masks import make_identity

F32 = mybir.dt.float32
BF16 = mybir.dt.bfloat16
AX = mybir.AxisListType
AF = mybir.ActivationFunctionType
ALU = mybir.AluOpType

B, L, N, D, E = 64, 336, 64, 32, 4
NB = 167          # rfft bins 1..167 (low_freq dropped, nyquist dropped)
B_PER = 8         # batch rows per core
NPAIR = B_PER // 2
NCORES = 8
LCH = 112         # L split into 3 chunks of 112
ND = N * D        # 2048
S2 = 2 * N        # 128 series per pair
KERNELS = (4, 8, 12)
PAD_F, PAD_E = 6, 5
CSLEN = 1 + PAD_F + L + PAD_E   # leading zero + padded series = 348
NOISE_EPS = 0.01
LOSS_COEF = 0.01
NEG_BIG = -1.0e30
JCH = (128, 39)   # NB split for reconstruction lhsT chunks


def _ap_with(a, pattern, extra_offset=0):
    return bass.AP(tensor=a.tensor, offset=a.offset + extra_offset, ap=pattern)


def build_nc():
    nc = bacc.Bacc("TRN2", target_bir_lowering=False, debug=False,
                   enable_asserts=False, num_devices=NCORES)

    x_ext = nc.dram_tensor("x", [B_PER, L, N, D], F32, kind="ExternalInput")
    w2_ext = nc.dram_tensor("w2", [S2, 2], F32, kind="ExternalInput")
    wg_ext = nc.dram_tensor("wg_w", [L, E], F32, kind="ExternalInput")
    wn_ext = nc.dram_tensor("wn_w", [L, E], F32, kind="ExternalInput")
    wgbe_ext = nc.dram_tensor("wg_be", [1, E], F32, kind="ExternalInput")
    wnbe_ext = nc.dram_tensor("wn_be", [1, E], F32, kind="ExternalInput")
    decw_ext = nc.dram_tensor("dec_w", [1, 3], F32, kind="ExternalInput")
    decb_ext = nc.dram_tensor("dec_b", [1, 3], F32, kind="ExternalInput")
    cf_ext = nc.dram_tensor("cf", [L, NB], F32, kind="ExternalInput")
    sfn_ext = nc.dram_tensor("sfn", [L, NB], F32, kind="ExternalInput")
    cr_ext = nc.dram_tensor("cr", [NB, L], BF16, kind="ExternalInput")
    srn_ext = nc.dram_tensor("srn", [NB, L], BF16, kind="ExternalInput")

    out_ext = nc.dram_tensor("out", [B_PER, L, N, D], F32, kind="ExternalOutput")
    stats_ext = nc.dram_tensor("stats", [2, NPAIR * 2 * E], F32, kind="ExternalOutput")

    # row b as [112, 6144]: partition p holds l = 3p..3p+2 (24 KB contiguous)
    x_rows = x_ext.ap().rearrange("b l n d -> b (l n d)")
    out_rows = out_ext.ap().rearrange("b l n d -> b (l n d)")

    with tile.TileContext(nc) as tc, ExitStack() as ctx:
        consts = ctx.enter_context(tc.tile_pool(name="consts", bufs=1))
        rowp = ctx.enter_context(tc.tile_pool(name="rowp", bufs=3))
        x3tp = ctx.enter_context(tc.tile_pool(name="x3tp", bufs=2))
        padp = ctx.enter_context(tc.tile_pool(name="padp", bufs=2))
        map_ = ctx.enter_context(tc.tile_pool(name="map", bufs=2))
        workp = ctx.enter_context(tc.tile_pool(name="workp", bufs=1))
        freqp = ctx.enter_context(tc.tile_pool(name="freqp", bufs=1))
        remtp = ctx.enter_context(tc.tile_pool(name="remtp", bufs=2))
        gtsp = ctx.enter_context(tc.tile_pool(name="gtsp", bufs=2))
        nanop = ctx.enter_context(tc.tile_pool(name="nanop", bufs=2))
        sbp = ctx.enter_context(tc.tile_pool(name="sbp", bufs=2))
        statp = ctx.enter_context(tc.tile_pool(name="statp", bufs=1))
        ps_small = ctx.enter_context(tc.tile_pool(name="ps_small", bufs=2, space="PSUM"))
        ps_dft = ctx.enter_context(tc.tile_pool(name="ps_dft", bufs=1, space="PSUM"))
        ps_seas = ctx.enter_context(tc.tile_pool(name="ps_seas", bufs=2, space="PSUM"))

        def emit_load(p):
            rows = (2 * p, 2 * p + 1)
            rowt = []
            for j in (0, 1):
                rt = rowp.tile([LCH, 3 * ND], F32, tag=f"row{j}")
                nc.sync.dma_start(rt[:], x_rows[rows[j]].rearrange("(p f) -> p f", p=LCH))
                rowt.append(rt)
            return rowt

        # issue the first loads before the constant DMAs so streaming starts
        # immediately
        loaded = {p: emit_load(p) for p in range(min(3, NPAIR))}

        # ---- constants ----
        identity = consts.tile([128, 128], F32, tag="ident")
        make_identity(nc, identity[:])
        identity_bf = consts.tile([128, 128], BF16, tag="identbf")
        make_identity(nc, identity_bf[:])

        cf_t, sfn_t, wg_t, wn_t = [], [], [], []
        for c in range(3):
            l0 = c * LCH
            t1 = consts.tile([LCH, NB], F32, tag=f"cf{c}")
            nc.sync.dma_start(t1[:], cf_ext.ap()[l0:l0 + LCH, :])
            cf_t.append(t1)
            t2 = consts.tile([LCH, NB], F32, tag=f"sfn{c}")
            nc.sync.dma_start(t2[:], sfn_ext.ap()[l0:l0 + LCH, :])
            sfn_t.append(t2)
            t3 = consts.tile([LCH, E], F32, tag=f"wg{c}")
            nc.sync.dma_start(t3[:], wg_ext.ap()[l0:l0 + LCH, :])
            wg_t.append(t3)
            t4 = consts.tile([LCH, E], F32, tag=f"wn{c}")
            nc.sync.dma_start(t4[:], wn_ext.ap()[l0:l0 + LCH, :])
            wn_t.append(t4)

        cr_t, srn_t = [], []
        j0 = 0
        for jc, jlen in enumerate(JCH):
            t1 = consts.tile([jlen, L], BF16, tag=f"cr{jc}")
            nc.sync.dma_start(t1[:], cr_ext.ap()[j0:j0 + jlen, :])
            cr_t.append(t1)
            t2 = consts.tile([jlen, L], BF16, tag=f"srn{jc}")
            nc.sync.dma_start(t2[:], srn_ext.ap()[j0:j0 + jlen, :])
            srn_t.append(t2)
            j0 += jlen

        w2_t = consts.tile([S2, 2], F32, tag="w2")
        nc.sync.dma_start(w2_t[:], w2_ext.ap())

        # [2, E] biases (both partitions hold the same row)
        wgbe_t = consts.tile([2, E], F32, tag="wgbe")
        a = wgbe_ext.ap()
        nc.sync.dma_start(wgbe_t[:], _ap_with(a, [[0, 2], a.ap[1]]))
        wnbe_t = consts.tile([2, E], F32, tag="wnbe")
        a = wnbe_ext.ap()
        nc.sync.dma_start(wnbe_t[:], _ap_with(a, [[0, 2], a.ap[1]]))

        decw_t = consts.tile([S2, 3], F32, tag="decw")
        a = decw_ext.ap()
        nc.sync.dma_start(decw_t[:], _ap_with(a, [[0, S2], a.ap[1]]))
        decb_t = consts.tile([S2, 3], F32, tag="decb")
        a = decb_ext.ap()
        nc.sync.dma_start(decb_t[:], _ap_with(a, [[0, S2], a.ap[1]]))

        stats_sb = statp.tile([2, NPAIR * 2 * E], F32, tag="stats")

        # ---- per-pair pipeline ----
        for p in range(NPAIR):
            rows = (2 * p, 2 * p + 1)
            rowt = loaded.pop(p) if p in loaded else emit_load(p)

            # x3T extraction: x3t[i][p, j*64+n] = rowt[j][p, i*2048 + n*32]
            # (chunk i partition p holds series values at l = 3p + i)
            x3t = []
            eng = [nc.gpsimd.tensor_copy, nc.gpsimd.tensor_copy, nc.gpsimd.tensor_copy]
            for i in range(3):
                t = x3tp.tile([LCH, S2], F32, tag=f"x3t{i}")
                for j in (0, 1):
                    src = rowt[j][:]
                    eng[i](t[:, j * N:(j + 1) * N],
                           _ap_with(src, [src.ap[0], [D, N]],
                                    extra_offset=i * ND))
                x3t.append(t)

            for j in (0, 1):
                dst = out_rows[rows[j]].rearrange("(p f) -> p f", p=LCH)
                if j == 0:
                    nc.gpsimd.dma_start(dst, rowt[j][:])
                else:
                    nc.scalar.dma_start(dst, rowt[j][:])

            # padbuf: [zero | 6 x front pad | x3 | 5 x end pad]; cumsum cs
            padbuf = padp.tile([S2, CSLEN], F32, tag="padbuf")
            nc.vector.memset(padbuf[:, 0:1], 0.0)
            pb = padbuf[:]
            cpeng = [nc.scalar.copy, nc.vector.tensor_copy, nc.scalar.copy]
            for i in range(3):
                pt = ps_small.tile([S2, LCH], F32, tag="ps_sm")
                nc.tensor.transpose(pt[:], x3t[i][:], identity[0:LCH, 0:LCH])
                cpeng[i](_ap_with(pb, [pb.ap[0], [3, LCH]],
                                  extra_offset=1 + PAD_F + i),
                         pt[:])
            x3a = padbuf[:, 1 + PAD_F:1 + PAD_F + L]
            nc.scalar.copy(padbuf[:, 1:1 + PAD_F],
                           padbuf[:, 1 + PAD_F:2 + PAD_F].to_broadcast([S2, PAD_F]))
            nc.scalar.copy(padbuf[:, 1 + PAD_F + L:CSLEN],
                           padbuf[:, PAD_F + L:1 + PAD_F + L].to_broadcast([S2, PAD_E]))
            cs = padp.tile([S2, CSLEN], F32, tag="cs")
            nc.vector.tensor_tensor_scan(cs[:], padbuf[:], padbuf[:], 0.0,
                                         op0=ALU.add, op1=ALU.bypass)

            # decomp softmax mix; ma_k = (cs[p0+k]-cs[p0])/k, p0 = l+6-front_k
            eds = []
            for d in range(3):
                ed = map_.tile([S2, L], F32, tag=f"ed{d}")
                nc.scalar.activation(ed[:], x3a, AF.Exp,
                                     bias=decb_t[:, d:d + 1], scale=decw_t[:, d:d + 1])
                eds.append(ed)
            den = workp.tile([S2, L], F32, tag="den")
            nc.vector.tensor_add(den[:], eds[0][:], eds[1][:])
            nc.vector.tensor_add(den[:], den[:], eds[2][:])
            recip = workp.tile([S2, L], F32, tag="recip")
            nc.vector.reciprocal_approx_fast(recip[:], den[:])

            num = workp.tile([S2, L], F32, tag="num")
            tmp = workp.tile([S2, L], F32, tag="tmpnum")
            for d, k in enumerate(KERNELS):
                front = (k - 1) - (k - 1) // 2
                p0 = PAD_F - front
                wsum = workp.tile([S2, L], F32, tag=f"wsum{d}")
                nc.vector.tensor_sub(wsum[:], cs[:, p0 + k:p0 + k + L],
                                     cs[:, p0:p0 + L])
                dst = num if d == 0 else tmp
                nc.vector.scalar_tensor_tensor(
                    out=dst[:], in0=eds[d][:], scalar=1.0 / k, in1=wsum[:],
                    op0=ALU.mult, op1=ALU.mult)
                if d > 0:
                    nc.vector.tensor_add(num[:], num[:], tmp[:])
            trend = workp.tile([S2, L], F32, tag="trend")
            nc.vector.tensor_mul(trend[:], num[:], recip[:])

            # forward DFT: RE/IM [128, 167]
            re_ps = ps_dft.tile([S2, NB], F32, tag="ps_re")
            im_ps = ps_dft.tile([S2, NB], F32, tag="ps_im")
            for c in range(3):
                nc.tensor.matmul(re_ps[:], x3t[c][:], cf_t[c][:],
                                 start=(c == 0), stop=(c == 2))
            for c in range(3):
                nc.tensor.matmul(im_ps[:], x3t[c][:], sfn_t[c][:],
                                 start=(c == 0), stop=(c == 2))

            # amp^2, top-3 mask, masked spectra
            sqre = freqp.tile([S2, NB], F32, tag="sqre")
            nc.scalar.activation(sqre[:], re_ps[:], AF.Square)
            sqim = freqp.tile([S2, NB], F32, tag="sqim")
            nc.scalar.activation(sqim[:], im_ps[:], AF.Square)
            amp2 = freqp.tile([S2, NB], F32, tag="amp2")
            nc.vector.tensor_add(amp2[:], sqre[:], sqim[:])
            m8 = freqp.tile([S2, 8], F32, tag="m8")
            nc.vector.max(out=m8[:], in_=amp2[:])
            mask = freqp.tile([S2, NB], F32, tag="mask")
            nc.vector.tensor_scalar(mask[:], amp2[:], m8[:, 2:3], None, op0=ALU.is_ge)
            rem = freqp.tile([S2, NB], BF16, tag="rem")
            nc.vector.tensor_mul(rem[:], re_ps[:], mask[:])
            imm = freqp.tile([S2, NB], BF16, tag="imm")
            nc.vector.tensor_mul(imm[:], im_ps[:], mask[:])

            # transpose masked spectra to [j, s]
            remt, immt = [], []
            j0 = 0
            for jc, jlen in enumerate(JCH):
                pt = ps_small.tile([jlen, S2], BF16, tag="ps_smb")
                nc.tensor.transpose(pt[:], rem[:, j0:j0 + jlen],
                                    identity_bf[0:S2, 0:S2])
                t = remtp.tile([jlen, S2], BF16, tag=f"remt{jc}")
                nc.scalar.copy(t[:], pt[:])
                remt.append(t)
                pt2 = ps_small.tile([jlen, S2], BF16, tag="ps_smb")
                nc.tensor.transpose(pt2[:], imm[:, j0:j0 + jlen],
                                    identity_bf[0:S2, 0:S2])
                t2 = remtp.tile([jlen, S2], BF16, tag=f"immt{jc}")
                nc.scalar.copy(t2[:], pt2[:])
                immt.append(t2)
                j0 += jlen

            # reconstruction
            seas_ps = ps_seas.tile([S2, L], F32, tag="ps_seas")
            nc.tensor.matmul(seas_ps[:], remt[0][:], cr_t[0][:], start=True, stop=False)
            nc.tensor.matmul(seas_ps[:], remt[1][:], cr_t[1][:], start=False, stop=False)
            nc.tensor.matmul(seas_ps[:], immt[0][:], srn_t[0][:], start=False, stop=False)
            nc.tensor.matmul(seas_ps[:], immt[1][:], srn_t[1][:], start=False, stop=True)

            # new_x = x3 + seas + trend
            newx0 = workp.tile([S2, L], F32, tag="newx0")
            nc.vector.tensor_add(newx0[:], seas_ps[:], trend[:])
            newx = workp.tile([S2, L], F32, tag="newx")
            nc.vector.tensor_add(newx[:], newx0[:], x3a)

            # gT chunks (block-diagonal start_w -> per-row g), gating GEMMs
            gts = []
            for c in range(3):
                pt = ps_small.tile([LCH, 2], F32, tag="ps_sm")
                nc.tensor.matmul(pt[:], newx[:, c * LCH:(c + 1) * LCH], w2_t[:],
                                 start=True, stop=True)
                t = gtsp.tile([LCH, 2], F32, tag=f"gts{c}")
                nc.scalar.copy(t[:], pt[:])
                gts.append(t)
            clean_ps = ps_small.tile([2, E], F32, tag="ps_sm")
            for c in range(3):
                nc.tensor.matmul(clean_ps[:], gts[c][:], wg_t[c][:],
                                 start=(c == 0), stop=(c == 2))
            wn_ps = ps_small.tile([2, E], F32, tag="ps_sm")
            for c in range(3):
                nc.tensor.matmul(wn_ps[:], gts[c][:], wn_t[c][:],
                                 start=(c == 0), stop=(c == 2))

            # nano stage [2, E]: stats + device gate-sum s (std == 1)
            clean = stats_sb[0:2, p * 2 * E:p * 2 * E + E]
            nc.vector.tensor_add(clean, clean_ps[:], wgbe_t[:])
            wnpre = stats_sb[0:2, p * 2 * E + E:p * 2 * E + 2 * E]
            nc.vector.tensor_add(wnpre, wn_ps[:], wnbe_t[:])
            nc.sync.dma_start(stats_ext.ap()[:, p * 2 * E:(p + 1) * 2 * E],
                              stats_sb[0:2, p * 2 * E:(p + 1) * 2 * E])



    nc.compile()
    return nc


def _dft_consts():
    j = np.arange(1, NB + 1, dtype=np.float64)
    t = np.arange(L, dtype=np.float64)
    ang = 2.0 * np.pi * np.outer(t, j) / L          # [L, NB]
    cf = np.cos(ang).astype(np.float32)
    sfn = (-np.sin(ang)).astype(np.float32)
    cr = ((2.0 / L) * np.cos(ang.T)).astype(ml_dtypes.bfloat16)   # [NB, L]
    srn = (-(2.0 / L) * np.sin(ang.T)).astype(ml_dtypes.bfloat16)
    return cf, sfn, cr, srn


_CACHE = {}


PERM = np.array([3 * p + i for i in range(3) for p in range(LCH)])


def make_in_maps(x, noise, start_w, start_b, wg_w, wg_b, wn_w, wn_b,
                 decomp_w, decomp_b):
    cf, sfn, cr, srn = _dft_consts()
    cf = np.ascontiguousarray(cf[PERM])
    sfn = np.ascontiguousarray(sfn[PERM])
    f32 = lambda v: np.ascontiguousarray(v, dtype=np.float32)
    w2 = np.zeros((S2, 2), np.float32)
    w2[0:N, 0] = np.asarray(start_w, np.float32)[:, 0]
    w2[N:S2, 1] = np.asarray(start_w, np.float32)[:, 0]
    wg_be = f32(wg_b + start_b[0] * wg_w.sum(0))[None, :]
    wn_be = f32(wn_b + start_b[0] * wn_w.sum(0))[None, :]
    common = {
        "w2": w2,
        "wg_w": f32(wg_w),
        "wn_w": f32(wn_w),
        "wg_be": wg_be,
        "wn_be": wn_be,
        "dec_w": f32(decomp_w).reshape(1, 3),
        "dec_b": f32(decomp_b).reshape(1, 3),
        "cf": cf, "sfn": sfn, "cr": cr, "srn": srn,
    }
    in_maps = []
    for i in range(NCORES):
        m = dict(common)
        m["x"] = f32(x[i * B_PER:(i + 1) * B_PER])
        in_maps.append(m)
    return in_maps


def unpack_stats(stats_list):
    """stats [2, NPAIR*8] per core -> clean [B, E], wnpre [B, E]."""
    clean = np.zeros((B, E), np.float32)
    wnpre = np.zeros((B, E), np.float32)
    for i, st in enumerate(stats_list):
        st = st.reshape(2, NPAIR, 2 * E)
        for p in range(NPAIR):
            for j in (0, 1):
                r = i * B_PER + 2 * p + j
                clean[r] = st[j, p, :E]
                wnpre[r] = st[j, p, E:]
    return clean, wnpre


def host_tail(stats_list, noise):
    clean32, wnpre32 = unpack_stats(stats_list)
    clean = clean32.astype(np.float64)
    std = np.log1p(np.exp(wnpre32.astype(np.float64))) + NOISE_EPS
    noisy = clean + noise.astype(np.float64) * std

    order = np.argsort(-noisy, axis=1, kind="stable")
    v1 = np.take_along_axis(noisy, order[:, 0:1], 1)
    v2 = np.take_along_axis(noisy, order[:, 1:2], 1)
    v3 = np.take_along_axis(noisy, order[:, 2:3], 1)
    e2 = np.exp(v2 - v1)
    den = 1.0 + e2
    gates = np.zeros((B, E), np.float64)
    np.put_along_axis(gates, order[:, 0:1], 1.0 / den, 1)
    np.put_along_axis(gates, order[:, 1:2], e2 / den, 1)
    imp = gates.sum(0)

    nerf = np.vectorize(math.erf)
    phi = lambda z: 0.5 * (1.0 + nerf(z / math.sqrt(2.0)))
    prob = np.where(noisy > v3, phi((clean - v3) / std), phi((clean - v2) / std))
    load = prob.sum(0)

    def cv2(v):
        return v.var(ddof=1) / (v.mean() ** 2 + 1e-10)

    return np.float32(LOSS_COEF * (cv2(imp) + cv2(load)))


def kernel(**inputs):
    if "nc" not in _CACHE:
        _CACHE["nc"] = build_nc()
    nc = _CACHE["nc"]

    inputs = {k: np.asarray(v) for k, v in inputs.items()}
    inputs.pop("padding_mask", None)
    in_maps = make_in_maps(**inputs)

    res = run_bass_kernel_spmd(nc, in_maps, core_ids=list(range(NCORES)))

    out = np.empty((B, L, N, D), np.float32)
    stats_list = []
    for i in range(NCORES):
        out[i * B_PER:(i + 1) * B_PER] = res.results[i]["out"].reshape(
            B_PER, L, N, D)
        stats_list.append(res.results[i]["stats"])
    loss = host_tail(stats_list, inputs["noise"])
    return out, loss


# revision 25
# speedup vs baseline: 1.1874x; 1.0157x over previous
"""Trainium2 Bass kernel for nn_AMS_18975165514201 (moe_routing).

Data-parallel over batch B=64 on 8 NeuronCores (8 rows/core), no
collectives.  Rows are processed in PAIRS so the series dimension packs
the full 128 partitions (2 rows x 64 channels).  Per pair, on device:
  - stream both rows of x through SBUF as [112, 6144] tiles (partition
    p holds l = 3p..3p+2, a contiguous 24 KB DRAM run) and store them
    straight back out: the SparseDispatcher combine with identity
    experts multiplies x by the row's top-2 softmax gate-sum, which is
    exactly 1 (softmax rows sum to 1; the reference's float gate-sums
    are 1 +- 1e-7), so output == x to ~6e-8 relative
  - extract x3 = x[..., 0] with strided on-chip copies -> [112, 128]
    x3T chunks (chunk i <-> l = 3p+i; DFT constants are row-permuted
    on the host to match)
  - series_decomp_multi: one cumsum (tensor_tensor_scan) over an
    edge-padded tile, window sums by shifted subtraction, mixed by a
    softmax of decomp projections (trend)
  - Fourier layer: fp32 DFT via matmuls against cos/sin constants,
    per-series top-3 bin selection (InstMax top-8 + is_ge mask),
    masked reconstruction in bf16 (selection stays fp32; bf16 here
    shifts clean logits < 1e-3, verified safe for the top-k choices)
  - gating GEMMs (block-diagonal start_w packs both rows) -> per-row
    clean logits and pre-softplus noise projections ("stats" output)
The host combine (the unshard step) computes noise_std / noisy top-k /
importance / load / balance loss from the 8 stats floats per row in
float64.  HW exec ~172 us vs a ~129 us pure-streaming envelope
(44 MB at the throttled ~340 GB/s per-core HBM rate).
"""

import math
from contextlib import ExitStack

import ml_dtypes
import numpy as np

import concourse.bass as bass
import concourse.tile as tile
from concourse import bacc, mybir
from concourse.bass_utils import run_bass_kernel_spmd
from concourse.masks import make_identity

F32 = mybir.dt.float32
BF16 = mybir.dt.bfloat16
AX = mybir.AxisListType
AF = mybir.ActivationFunctionType
ALU = mybir.AluOpType

B, L, N, D, E = 64, 336, 64, 32, 4
NB = 167          # rfft bins 1..167 (low_freq dropped, nyquist dropped)
B_PER = 8         # batch rows per core
NPAIR = B_PER // 2
NCORES = 8
LCH = 112         # L split into 3 chunks of 112
ND = N * D        # 2048
S2 = 2 * N        # 128 series per pair
KERNELS = (4, 8, 12)
PAD_F, PAD_E = 6, 5
CSLEN = 1 + PAD_F + L + PAD_E   # leading zero + padded series = 348
NOISE_EPS = 0.01
LOSS_COEF = 0.01
NEG_BIG = -1.0e30
JCH = (128, 39)   # NB split for reconstruction lhsT chunks


def _ap_with(a, pattern, extra_offset=0):
    return bass.AP(tensor=a.tensor, offset=a.offset + extra_offset, ap=pattern)


def build_nc():
    nc = bacc.Bacc("TRN2", target_bir_lowering=False, debug=False,
                   enable_asserts=False, num_devices=NCORES)

    x_ext = nc.dram_tensor("x", [B_PER, L, N, D], F32, kind="ExternalInput")
    w2_ext = nc.dram_tensor("w2", [S2, 2], F32, kind="ExternalInput")
    wg_ext = nc.dram_tensor("wg_w", [L, E], F32, kind="ExternalInput")
    wn_ext = nc.dram_tensor("wn_w", [L, E], F32, kind="ExternalInput")
    wgbe_ext = nc.dram_tensor("wg_be", [1, E], F32, kind="ExternalInput")
    wnbe_ext = nc.dram_tensor("wn_be", [1, E], F32, kind="ExternalInput")
    decw_ext = nc.dram_tensor("dec_w", [1, 3], F32, kind="ExternalInput")
    decb_ext = nc.dram_tensor("dec_b", [1, 3], F32, kind="ExternalInput")
    cf_ext = nc.dram_tensor("cf", [L, NB], F32, kind="ExternalInput")
    sfn_ext = nc.dram_tensor("sfn", [L, NB], F32, kind="ExternalInput")
    cr_ext = nc.dram_tensor("cr", [NB, L], BF16, kind="ExternalInput")
    srn_ext = nc.dram_tensor("srn", [NB, L], BF16, kind="ExternalInput")

    out_ext = nc.dram_tensor("out", [B_PER, L, N, D], F32, kind="ExternalOutput")
    stats_ext = nc.dram_tensor("stats", [2, NPAIR * 2 * E], F32, kind="ExternalOutput")

    # row b as [112, 6144]: partition p holds l = 3p..3p+2 (24 KB contiguous)
    x_rows = x_ext.ap().rearrange("b l n d -> b (l n d)")
    out_rows = out_ext.ap().rearrange("b l n d -> b (l n d)")

    with tile.TileContext(nc) as tc, ExitStack() as ctx:
        consts = ctx.enter_context(tc.tile_pool(name="consts", bufs=1))
        rowp = ctx.enter_context(tc.tile_pool(name="rowp", bufs=3))
        x3tp = ctx.enter_context(tc.tile_pool(name="x3tp", bufs=2))
        padp = ctx.enter_context(tc.tile_pool(name="padp", bufs=2))
        map_ = ctx.enter_context(tc.tile_pool(name="map", bufs=2))
        workp = ctx.enter_context(tc.tile_pool(name="workp", bufs=1))
        freqp = ctx.enter_context(tc.tile_pool(name="freqp", bufs=1))
        remtp = ctx.enter_context(tc.tile_pool(name="remtp", bufs=2))
        gtsp = ctx.enter_context(tc.tile_pool(name="gtsp", bufs=2))
        nanop = ctx.enter_context(tc.tile_pool(name="nanop", bufs=2))
        sbp = ctx.enter_context(tc.tile_pool(name="sbp", bufs=2))
        statp = ctx.enter_context(tc.tile_pool(name="statp", bufs=1))
        ps_small = ctx.enter_context(tc.tile_pool(name="ps_small", bufs=2, space="PSUM"))
        ps_dft = ctx.enter_context(tc.tile_pool(name="ps_dft", bufs=1, space="PSUM"))
        ps_seas = ctx.enter_context(tc.tile_pool(name="ps_seas", bufs=2, space="PSUM"))

        def emit_load(p):
            rows = (2 * p, 2 * p + 1)
            rowt = []
            for j in (0, 1):
                rt = rowp.tile([LCH, 3 * ND], F32, tag=f"row{j}")
                nc.sync.dma_start(rt[:], x_rows[rows[j]].rearrange("(p f) -> p f", p=LCH))
                rowt.append(rt)
            return rowt

        # issue the first loads before the constant DMAs so streaming starts
        # immediately
        loaded = {p: emit_load(p) for p in range(min(3, NPAIR))}

        # ---- constants ----
        identity = consts.tile([128, 128], F32, tag="ident")
        make_identity(nc, identity[:])
        identity_bf = consts.tile([128, 128], BF16, tag="identbf")
        make_identity(nc, identity_bf[:])

        cf_t, sfn_t, wg_t, wn_t = [], [], [], []
        for c in range(3):
            l0 = c * LCH
            t1 = consts.tile([LCH, NB], F32, tag=f"cf{c}")
            nc.sync.dma_start(t1[:], cf_ext.ap()[l0:l0 + LCH, :])
            cf_t.append(t1)
            t2 = consts.tile([LCH, NB], F32, tag=f"sfn{c}")
            nc.sync.dma_start(t2[:], sfn_ext.ap()[l0:l0 + LCH, :])
            sfn_t.append(t2)
            t3 = consts.tile([LCH, E], F32, tag=f"wg{c}")
            nc.sync.dma_start(t3[:], wg_ext.ap()[l0:l0 + LCH, :])
            wg_t.append(t3)
            t4 = consts.tile([LCH, E], F32, tag=f"wn{c}")
            nc.sync.dma_start(t4[:], wn_ext.ap()[l0:l0 + LCH, :])
            wn_t.append(t4)

        cr_t, srn_t = [], []
        j0 = 0
        for jc, jlen in enumerate(JCH):
            t1 = consts.tile([jlen, L], BF16, tag=f"cr{jc}")
            nc.sync.dma_start(t1[:], cr_ext.ap()[j0:j0 + jlen, :])
            cr_t.append(t1)
            t2 = consts.tile([jlen, L], BF16, tag=f"srn{jc}")
            nc.sync.dma_start(t2[:], srn_ext.ap()[j0:j0 + jlen, :])
            srn_t.append(t2)
            j0 += jlen

        w2_t = consts.tile([S2, 2], F32, tag="w2")
        nc.sync.dma_start(w2_t[:], w2_ext.ap())

        # [2, E] biases (both partitions hold the same row)
        wgbe_t = consts.tile([2, E], F32, tag="wgbe")
        a = wgbe_ext.ap()
        nc.sync.dma_start(wgbe_t[:], _ap_with(a, [[0, 2], a.ap[1]]))
        wnbe_t = consts.tile([2, E], F32, tag="wnbe")
        a = wnbe_ext.ap()
        nc.sync.dma_start(wnbe_t[:], _ap_with(a, [[0, 2], a.ap[1]]))

        decw_t = consts.tile([S2, 3], F32, tag="decw")
        a = decw_ext.ap()
        nc.sync.dma_start(decw_t[:], _ap_with(a, [[0, S2], a.ap[1]]))
        decb_t = consts.tile([S2, 3], F32, tag="decb")
        a = decb_ext.ap()
        nc.sync.dma_start(decb_t[:], _ap_with(a, [[0, S2], a.ap[1]]))

        stats_sb = statp.tile([2, NPAIR * 2 * E], F32, tag="stats")

        # ---- per-pair pipeline ----
        for p in range(NPAIR):
            rows = (2 * p, 2 * p + 1)
            rowt = loaded.pop(p) if p in loaded else emit_load(p)

            # x3T extraction: x3t[i][p, j*64+n] = rowt[j][p, i*2048 + n*32]
            # (chunk i partition p holds series values at l = 3p + i)
            x3t = []
            eng = [nc.gpsimd.tensor_copy, nc.gpsimd.tensor_copy, nc.gpsimd.tensor_copy]
            for i in range(3):
                t = x3tp.tile([LCH, S2], F32, tag=f"x3t{i}")
                for j in (0, 1):
                    src = rowt[j][:]
                    eng[i](t[:, j * N:(j + 1) * N],
                           _ap_with(src, [src.ap[0], [D, N]],
                                    extra_offset=i * ND))
                x3t.append(t)

            for j in (0, 1):
                dst = out_rows[rows[j]].rearrange("(p f) -> p f", p=LCH)
                if j == 0:
                    nc.gpsimd.dma_start(dst, rowt[j][:])
                else:
                    nc.scalar.dma_start(dst, rowt[j][:])

            # padbuf: [zero | 6 x front pad | x3 | 5 x end pad]; cumsum cs
            padbuf = padp.tile([S2, CSLEN], F32, tag="padbuf")
            nc.vector.memset(padbuf[:, 0:1], 0.0)
            pb = padbuf[:]
            cpeng = [nc.scalar.copy, nc.vector.tensor_copy, nc.scalar.copy]
            for i in range(3):
                pt = ps_small.tile([S2, LCH], F32, tag="ps_sm")
                nc.tensor.transpose(pt[:], x3t[i][:], identity[0:LCH, 0:LCH])
                cpeng[i](_ap_with(pb, [pb.ap[0], [3, LCH]],
                                  extra_offset=1 + PAD_F + i),
                         pt[:])
            x3a = padbuf[:, 1 + PAD_F:1 + PAD_F + L]
            nc.scalar.copy(padbuf[:, 1:1 + PAD_F],
                           padbuf[:, 1 + PAD_F:2 + PAD_F].to_broadcast([S2, PAD_F]))
            nc.scalar.copy(padbuf[:, 1 + PAD_F + L:CSLEN],
                           padbuf[:, PAD_F + L:1 + PAD_F + L].to_broadcast([S2, PAD_E]))
            cs = padp.tile([S2, CSLEN], F32, tag="cs")
            nc.vector.tensor_tensor_scan(cs[:], padbuf[:], padbuf[:], 0.0,
                                         op0=ALU.add, op1=ALU.bypass)

            # decomp softmax mix; ma_k = (cs[p0+k]-cs[p0])/k, p0 = l+6-front_k
            eds = []
            for d in range(3):
                ed = map_.tile([S2, L], F32, tag=f"ed{d}")
                nc.scalar.activation(ed[:], x3a, AF.Exp,
                                     bias=decb_t[:, d:d + 1], scale=decw_t[:, d:d + 1])
                eds.append(ed)
            den = workp.tile([S2, L], F32, tag="den")
            nc.vector.tensor_add(den[:], eds[0][:], eds[1][:])
            nc.vector.tensor_add(den[:], den[:], eds[2][:])
            recip = workp.tile([S2, L], F32, tag="recip")
            nc.vector.reciprocal_approx_fast(recip[:], den[:])

            num = workp.tile([S2, L], F32, tag="num")
            tmp = workp.tile([S2, L], F32, tag="tmpnum")
            for d, k in enumerate(KERNELS):
                front = (k - 1) - (k - 1) // 2
                p0 = PAD_F - front
                wsum = workp.tile([S2, L], F32, tag=f"wsum{d}")
                nc.vector.tensor_sub(wsum[:], cs[:, p0 + k:p0 + k + L],
                                     cs[:, p0:p0 + L])
                dst = num if d == 0 else tmp
                nc.vector.scalar_tensor_tensor(
                    out=dst[:], in0=eds[d][:], scalar=1.0 / k, in1=wsum[:],
                    op0=ALU.mult, op1=ALU.mult)
                if d > 0:
                    nc.vector.tensor_add(num[:], num[:], tmp[:])
            trend = workp.tile([S2, L], F32, tag="trend")
            nc.vector.tensor_mul(trend[:], num[:], recip[:])

            # forward DFT: RE/IM [128, 167]
            re_ps = ps_dft.tile([S2, NB], F32, tag="ps_re")
            im_ps = ps_dft.tile([S2, NB], F32, tag="ps_im")
            for c in range(3):
                nc.tensor.matmul(re_ps[:], x3t[c][:], cf_t[c][:],
                                 start=(c == 0), stop=(c == 2))
            for c in range(3):
                nc.tensor.matmul(im_ps[:], x3t[c][:], sfn_t[c][:],
                                 start=(c == 0), stop=(c == 2))

            # amp^2, top-3 mask, masked spectra
            sqre = freqp.tile([S2, NB], F32, tag="sqre")
            nc.scalar.activation(sqre[:], re_ps[:], AF.Square)
            sqim = freqp.tile([S2, NB], F32, tag="sqim")
            nc.scalar.activation(sqim[:], im_ps[:], AF.Square)
            amp2 = freqp.tile([S2, NB], F32, tag="amp2")
            nc.vector.tensor_add(amp2[:], sqre[:], sqim[:])
            m8 = freqp.tile([S2, 8], F32, tag="m8")
            nc.vector.max(out=m8[:], in_=amp2[:])
            mask = freqp.tile([S2, NB], F32, tag="mask")
            nc.vector.tensor_scalar(mask[:], amp2[:], m8[:, 2:3], None, op0=ALU.is_ge)
            rem = freqp.tile([S2, NB], BF16, tag="rem")
            nc.vector.tensor_mul(rem[:], re_ps[:], mask[:])
            imm = freqp.tile([S2, NB], BF16, tag="imm")
            nc.vector.tensor_mul(imm[:], im_ps[:], mask[:])

            # transpose masked spectra to [j, s]
            remt, immt = [], []
            j0 = 0
            for jc, jlen in enumerate(JCH):
                pt = ps_small.tile([jlen, S2], BF16, tag="ps_smb")
                nc.tensor.transpose(pt[:], rem[:, j0:j0 + jlen],
                                    identity_bf[0:S2, 0:S2])
                t = remtp.tile([jlen, S2], BF16, tag=f"remt{jc}")
                nc.scalar.copy(t[:], pt[:])
                remt.append(t)
                pt2 = ps_small.tile([jlen, S2], BF16, tag="ps_smb")
                nc.tensor.transpose(pt2[:], imm[:, j0:j0 + jlen],
                                    identity_bf[0:S2, 0:S2])
                t2 = remtp.tile([jlen, S2], BF16, tag=f"immt{jc}")
                nc.scalar.copy(t2[:], pt2[:])
                immt.append(t2)
                j0 += jlen

            # reconstruction
            seas_ps = ps_seas.tile([S2, L], F32, tag="ps_seas")
            nc.tensor.matmul(seas_ps[:], remt[0][:], cr_t[0][:], start=True, stop=False)
            nc.tensor.matmul(seas_ps[:], remt[1][:], cr_t[1][:], start=False, stop=False)
            nc.tensor.matmul(seas_ps[:], immt[0][:], srn_t[0][:], start=False, stop=False)
            nc.tensor.matmul(seas_ps[:], immt[1][:], srn_t[1][:], start=False, stop=True)

            # new_x = x3 + seas + trend
            newx0 = workp.tile([S2, L], F32, tag="newx0")
            nc.vector.tensor_add(newx0[:], seas_ps[:], trend[:])
            newx = workp.tile([S2, L], F32, tag="newx")
            nc.vector.tensor_add(newx[:], newx0[:], x3a)

            # gT chunks (block-diagonal start_w -> per-row g), gating GEMMs
            gts = []
            for c in range(3):
                pt = ps_small.tile([LCH, 2], F32, tag="ps_sm")
                nc.tensor.matmul(pt[:], newx[:, c * LCH:(c + 1) * LCH], w2_t[:],
                                 start=True, stop=True)
                t = gtsp.tile([LCH, 2], F32, tag=f"gts{c}")
                nc.scalar.copy(t[:], pt[:])
                gts.append(t)
            clean_ps = ps_small.tile([2, E], F32, tag="ps_sm")
            for c in range(3):
                nc.tensor.matmul(clean_ps[:], gts[c][:], wg_t[c][:],
                                 start=(c == 0), stop=(c == 2))
            wn_ps = ps_small.tile([2, E], F32, tag="ps_sm")
            for c in range(3):
                nc.tensor.matmul(wn_ps[:], gts[c][:], wn_t[c][:],
                                 start=(c == 0), stop=(c == 2))

            # nano stage [2, E]: stats + device gate-sum s (std == 1)
            clean = stats_sb[0:2, p * 2 * E:p * 2 * E + E]
            nc.vector.tensor_add(clean, clean_ps[:], wgbe_t[:])
            wnpre = stats_sb[0:2, p * 2 * E + E:p * 2 * E + 2 * E]
            nc.vector.tensor_add(wnpre, wn_ps[:], wnbe_t[:])
            nc.sync.dma_start(stats_ext.ap()[:, p * 2 * E:(p + 1) * 2 * E],
                              stats_sb[0:2, p * 2 * E:(p + 1) * 2 * E])



    nc.compile()
    return nc


def _dft_consts():
    j = np.arange(1, NB + 1, dtype=np.float64)
    t = np.arange(L, dtype=np.float64)
    ang = 2.0 * np.pi * np.outer(t, j) / L          # [L, NB]
    cf = np.cos(ang).astype(np.float32)
    sfn = (-np.sin(ang)).astype(np.float32)
    cr = ((2.0 / L) * np.cos(ang.T)).astype(ml_dtypes.bfloat16)   # [NB, L]
    srn = (-(2.0 / L) * np.sin(ang.T)).astype(ml_dtypes.bfloat16)
    return cf, sfn, cr, srn


_CACHE = {}


PERM = np.array([3 * p + i for i in range(3) for p in range(LCH)])


def make_in_maps(x, noise, start_w, start_b, wg_w, wg_b, wn_w, wn_b,
                 decomp_w, decomp_b):
    cf, sfn, cr, srn = _dft_consts()
    cf = np.ascontiguousarray(cf[PERM])
    sfn = np.ascontiguousarray(sfn[PERM])
    f32 = lambda v: np.ascontiguousarray(v, dtype=np.float32)
    w2 = np.zeros((S2, 2), np.float32)
    w2[0:N, 0] = np.asarray(start_w, np.float32)[:, 0]
    w2[N:S2, 1] = np.asarray(start_w, np.float32)[:, 0]
    wg_be = f32(wg_b + start_b[0] * wg_w.sum(0))[None, :]
    wn_be = f32(wn_b + start_b[0] * wn_w.sum(0))[None, :]
    common = {
        "w2": w2,
        "wg_w": f32(wg_w),
        "wn_w": f32(wn_w),
        "wg_be": wg_be,
        "wn_be": wn_be,
        "dec_w": f32(decomp_w).reshape(1, 3),
        "dec_b": f32(decomp_b).reshape(1, 3),
        "cf": cf, "sfn": sfn, "cr": cr, "srn": srn,
    }
    in_maps = []
    for i in range(NCORES):
        m = dict(common)
        m["x"] = f32(x[i * B_PER:(i + 1) * B_PER])
        in_maps.append(m)
    return in_maps


def unpack_stats(stats_list):
    """stats [2, NPAIR*8] per core -> clean [B, E], wnpre [B, E]."""
    clean = np.zeros((B, E), np.float32)
    wnpre = np.zeros((B, E), np.float32)
    for i, st in enumerate(stats_list):
        st = st.reshape(2, NPAIR, 2 * E)
        for p in range(NPAIR):
            for j in (0, 1):
                r = i * B_PER + 2 * p + j
                clean[r] = st[j, p, :E]
                wnpre[r] = st[j, p, E:]
    return clean, wnpre


def host_tail(stats_list, noise):
    clean32, wnpre32 = unpack_stats(stats_list)
    clean = clean32.astype(np.float64)
    std = np.log1p(np.exp(wnpre32.astype(np.float64))) + NOISE_EPS
    noisy = clean + noise.astype(np.float64) * std

    order = np.argsort(-noisy, axis=1, kind="stable")
    v1 = np.take_along_axis(noisy, order[:, 0:1], 1)
    v2 = np.take_along_axis(noisy, order[:, 1:2], 1)
    v3 = np.take_along_axis(noisy, order[:, 2:3], 1)
    e2 = np.exp(v2 - v1)
    den = 1.0 + e2
    gates = np.zeros((B, E), np.float64)
    np.put_along_axis(gates, order[:, 0:1], 1.0 / den, 1)
    np.put_along_axis(gates, order[:, 1:2], e2 / den, 1)
    imp = gates.sum(0)

    nerf = np.vectorize(math.erf)
    phi = lambda z: 0.5 * (1.0 + nerf(z / math.sqrt(2.0)))
    prob = np.where(noisy > v3, phi((clean - v3) / std), phi((clean - v2) / std))
    load = prob.sum(0)

    def cv2(v):
        return v.var(ddof=1) / (v.mean() ** 2 + 1e-10)

    return np.float32(LOSS_COEF * (cv2(imp) + cv2(load)))


def kernel(**inputs):
    if "nc" not in _CACHE:
        _CACHE["nc"] = build_nc()
    nc = _CACHE["nc"]

    inputs = {k: np.asarray(v) for k, v in inputs.items()}
    inputs.pop("padding_mask", None)
    in_maps = make_in_maps(**inputs)

    res = run_bass_kernel_spmd(nc, in_maps, core_ids=list(range(NCORES)))

    out = np.empty((B, L, N, D), np.float32)
    stats_list = []
    for i in range(NCORES):
        out[i * B_PER:(i + 1) * B_PER] = res.results[i]["out"].reshape(
            B_PER, L, N, D)
        stats_list.append(res.results[i]["stats"])
    loss = host_tail(stats_list, inputs["noise"])
    return out, loss
